# revision 31
# baseline (speedup 1.0000x reference)
"""Trainium2 Bass kernel for nn_GuidedCorrespondenceLoss (fp8 DoubleRow).

Strategy (8 NeuronCores, SPMD, target rows sharded 512/core):
  - Host: nearest-neighbor fields -> patch indices; gather 7x7x128 patch
    columns; center by refer-mean and normalize in f64; quantize the
    normalized features to fp8 e4m3 scaled by A=128. The cosine-distance
    matmul runs entirely in fp8 DoubleRow mode (2 contraction rows per
    partition): Pf = F*d_cos with F = 2*A^2 = 32768, via 25 DR k-pairs
    (6272 feat rows + one 128*128 const row pair = 0.5F + zero padding).
  - PE microarch rules (measured): a matmul whose moving operand exceeds
    512 bytes/partition stalls back-to-back streaming (~1.2 cy/col vs
    1.0), and two accumulation chains sharing one PSUM bank serialize.
    So: 256-col fp8 DR chunks (2B/partition/col), column groups of 512
    with 8 concurrent chains in 8 distinct banks (half-bank used,
    ping-pong halves across groups to hide the drain).
  - Mask L2 term (reference broadcast quirk: d_prog[i,j] = tpn_i + rpn_j
    - 2*tp_j.rp_i, clipped at 0 -- the clip is materially active) runs as
    an upfront bf16 phase: Pm = rp^T tp - 0.5*(rpn_hi + rpn_lo) with the
    rpn hi/lo split giving ~1e-5 relative precision on the const term.
    v' = relu(-(20F/49)*Pm + (10F/49)*tpn_i) on ACT writes d_t directly.
  - Feature chains accumulate in PSUM per (m-tile, 256-col chunk); d_t +=
    Pf on DVE; per-(m,group) partial min; per-m: dmin, exp row-sum
    (relaxed contextual loss stats).
  - Host: loss_i = log(sumw_i) - 2*(1 - dmin_i/(dmin_i+EPS)); mean.

Numerics validated host-side: rel err ~1.4e-4 vs f64 reference (gate 2e-2).
"""
import numpy as np
import ml_dtypes

EPS = 2.220446049250313e-16
PATCH, STRIDE, SAMPLE, H_BW, PROG_W = 7, 3, 64, 0.5, 10.0
HO = (256 - PATCH) // STRIDE + 1     # 84

N_CORES = 8
NT = 4096              # total target rows
TPC = NT // N_CORES    # 512 rows per core
MT = TPC // 128        # 4 m-tiles per core
NR = 4096              # refer columns
GRP = 256              # columns per feature group
NG = NR // GRP         # 16 groups
BANK = 512             # psum bank width (f32)
CHK = 256              # matmul moving-chunk columns
K = PATCH * PATCH      # 49
CH = 128
KC = CH * K            # 6272 contraction rows
KP = 25                # DoubleRow k-pairs (6400 rows padded)
KCP = KP * 256         # 6400
KM = K + 2             # mask chain contraction (49 + rpn hi/lo rows)
FSC = 32768.0          # F = 2 * A^2
ASC = 128.0            # feature quantization scale

E4NP = ml_dtypes.float8_e4m3
BFNP = ml_dtypes.bfloat16

_PROGRAM_CACHE = {}
import os
VARIANT = os.environ.get("KERNEL_VARIANT", "full")


def _field_to_idx(field):
    g = np.asarray(field)[0].reshape(-1, 2).astype(np.float32)
    gx = (g[:, 0] + np.float32(1.0)) * np.float32(0.5) * np.float32(HO - 1)
    gy = (g[:, 1] + np.float32(1.0)) * np.float32(0.5) * np.float32(HO - 1)
    ix = np.clip(np.round(gx), 0, HO - 1).astype(np.int64)
    iy = np.clip(np.round(gy), 0, HO - 1).astype(np.int64)
    return iy, ix


def _gather_cols(feat_chw, iy, ix):
    """feat [C,256,256] -> [C, 49, n]: out[c,k,j] = feat[c, 3*iy+kh, 3*ix+kw]."""
    iy3, ix3 = iy * STRIDE, ix * STRIDE
    kh = np.repeat(np.arange(PATCH), PATCH)
    kw = np.tile(np.arange(PATCH), PATCH)
    HH = iy3[None, :] + kh[:, None]
    WW = ix3[None, :] + kw[:, None]
    return feat_chw[:, HH, WW]


def _emit_rep(tc, nc, aps, env, d_t, sfx):
    from concourse import mybir

    f32 = mybir.dt.float32
    bf16 = mybir.dt.bfloat16
    fp8 = mybir.dt.float8e4
    AF = mybir.ActivationFunctionType
    OP = mybir.AluOpType
    DR = mybir.MatmulPerfMode.DoubleRow

    yk, souts = aps["yk"], aps["souts"]
    x_t, xm_t, ym_t, rowt_t, cst = (
        env["x_t"], env["xm_t"], env["ym_t"], env["rowt_t"], env["cst"])
    ypool, wpool, spool, ps = (
        env["ypool"], env["wpool"], env["spool"], env["ps"])

    minp = spool.tile([CH, MT * NG], f32, name=f"minp_{sfx}", tag="minp")

    # ---- per-group: mask block (4 banks rotating) + feature chains (4
    # dedicated full-width banks) + DVE drains.  The mask matmuls at each
    # group head cover the PE window while DVE drains the previous group's
    # chains; the DVE relu (mkl pre-scaled by -(2*PROG_W*FSC/K) on host:
    # d = max(Pm' + tpn'_i, 0)) keeps ACT free for phase C's exp, which
    # then overlaps the next rep's start.
    nodma = VARIANT == "mmnodma"
    KH = (KP + 1) // 2  # y DMA half-group granularity (13 + 12 k-pairs)
    pfs = [env["psf"].tile([CH, MT, GRP], f32, name=f"pf_{p}")
           for p in range(2)]
    for g in range(NG):
        pf = pfs[g % 2]
        if nodma:
            yh = env["yh_c"]
        else:
            yh = ypool.tile([CH, KP, 2, GRP], fp8, name=f"yh_{g}{sfx}",
                            tag="y")
            nc.sync.dma_start(
                yh[:], yk[:, (g * KP) * 2 * GRP: (g + 1) * KP * 2 * GRP])
        if VARIANT in ("full", "nophc"):
            for m in range(MT):
                pm = ps.tile([CH, GRP], f32, name=f"pm_{g}_{m}{sfx}",
                             tag="ps")
                nc.tensor.matmul(
                    pm[:],
                    lhsT=xm_t[:, :, m * 128:(m + 1) * 128],
                    rhs=ym_t[:, :, g * GRP:(g + 1) * GRP],
                    start=True, stop=True,
                    perf_mode=DR,
                )
                nc.vector.tensor_scalar(
                    out=d_t[:, m * NR + g * GRP: m * NR + (g + 1) * GRP],
                    in0=pm[:], scalar1=rowt_t[:, m:m + 1], scalar2=0.0,
                    op0=OP.add, op1=OP.max)
        if VARIANT == "dma":
            for m in range(MT):
                nc.tensor.matmul(
                    pf[:, m, :],
                    lhsT=x_t[:, 0:2, m * 128:(m + 1) * 128],
                    rhs=yh[:, 0, :, :],
                    start=True, stop=True,
                    perf_mode=DR,
                )
        else:
          kps = list(range(KP)) if VARIANT != "mmhalf" else list(range(0, KP, 2))
          for kp in kps:
            for m in range(MT):
                nc.tensor.matmul(
                    pf[:, m, :],
                    lhsT=x_t[:, 2 * kp:2 * kp + 2, m * 128:(m + 1) * 128],
                    rhs=yh[:, kp, :, :],
                    start=(kp == kps[0]), stop=(kp == kps[-1]),
                    perf_mode=DR,
                )
        for m in range(MT):
            dsl = d_t[:, m * NR + g * GRP: m * NR + (g + 1) * GRP]
            if VARIANT in ("full", "peadd", "nophc", "nomask"):
                nc.vector.tensor_tensor(
                    out=dsl[:], in0=dsl[:], in1=pf[:, m, :], op=OP.add)
                nc.vector.tensor_reduce(
                    minp[:, m * NG + g: m * NG + g + 1], dsl,
                    axis=mybir.AxisListType.X, op=OP.min)
            else:
                nc.vector.tensor_copy(
                    minp[:, m * NG + g: m * NG + g + 1], pf[:, m, 0:1])

    # ---- Phase C: row stats ---------------------------------------------
    so = spool.tile([CH, 2 * MT], f32, name=f"so_{sfx}", tag="so")
    if VARIANT not in ("full", "nomask"):
        nc.vector.tensor_copy(so[:, 0:MT], minp[:, 0:MT])
        nc.vector.tensor_copy(so[:, MT:2 * MT], minp[:, 0:MT])
        nc.vector.dma_start(souts[:], so[:])
        return
    # dmin and the exp row-sum land directly in the output tile: no DVE
    # copies behind the ACT accum, so the next rep's DVE work (and with it
    # the PE's PSUM rotation) never queues behind this rep's exp.
    for m in range(MT):
        dmin = so[:, 2 * m:2 * m + 1]
        nc.vector.tensor_reduce(dmin, minp[:, m * NG:(m + 1) * NG],
                                axis=mybir.AxisListType.X, op=OP.min)
        dme = spool.tile([CH, 1], f32, name=f"dme_{m}{sfx}", tag="st")
        nc.vector.tensor_scalar(
            out=dme[:], in0=dmin, scalar1=float(1.0 / FSC),
            scalar2=float(EPS), op0=OP.mult, op1=OP.add)
        rec = spool.tile([CH, 1], f32, name=f"rec_{m}{sfx}", tag="st")
        nc.vector.reciprocal(rec[:], dme[:])
        scl = spool.tile([CH, 1], f32, name=f"scl_{m}{sfx}", tag="st")
        nc.vector.tensor_scalar_mul(scl[:], rec[:], float(-2.0 / FSC))
        wtmp = wpool.tile([CH, NR], bf16, name=f"w_{m}{sfx}", tag="w")
        nc.scalar.activation(wtmp[:], d_t[:, m * NR:(m + 1) * NR], AF.Exp,
                             bias=cst[:, 0:1], scale=scl[:, 0:1],
                             accum_out=so[:, 2 * m + 1:2 * m + 2])

    # issue from ACT's queue: it waits on exp(m3) anyway, and must not
    # block the SP queue that streams the next rep's y slabs.
    nc.scalar.dma_start(souts[:], so[:])


def _build_kernel_body(tc, aps, loop_iters=1):
    from concourse import mybir

    nc = tc.nc
    f32 = mybir.dt.float32
    bf16 = mybir.dt.bfloat16
    fp8 = mybir.dt.float8e4

    with (
        tc.tile_pool(name="xpool", bufs=1) as xpool,
        tc.tile_pool(name="dpool", bufs=1) as dpool,
        tc.tile_pool(name="ypool", bufs=3) as ypool,
        tc.tile_pool(name="wpool", bufs=2) as wpool,
        tc.tile_pool(name="spool", bufs=4) as spool,
        tc.tile_pool(name="ps", bufs=4, space="PSUM") as ps,
        tc.tile_pool(name="psf", bufs=1, space="PSUM") as psf,
    ):
        # resident loads (outside the timing loop)
        x_t = xpool.tile([CH, KP * 2, TPC], fp8, name="x_t")
        nc.sync.dma_start(x_t[:], aps["xk"][:])
        xm_t = xpool.tile([CH, 2, TPC], fp8, name="xm_t")
        nc.sync.dma_start(xm_t[:], aps["xm"][:])
        ym_t = xpool.tile([CH, 2, NR], fp8, name="ym_t")
        nc.sync.dma_start(ym_t[:], aps["ym"][:])
        rowt_t = xpool.tile([CH, MT], f32, name="rowt_t")
        nc.sync.dma_start(rowt_t[:], aps["rowt"][:])
        cst = xpool.tile([CH, 1], f32, name="cst")
        nc.vector.memset(cst[:, 0:1], 2.0)

        d_A = dpool.tile([CH, MT * NR], bf16, name="d_A")
        d_B = dpool.tile([CH, MT * NR], bf16, name="d_B")

        if VARIANT == "mmnodma":
            yh_c = xpool.tile([CH, KP, 2, GRP], fp8, name="yh_c")
            nc.sync.dma_start(yh_c[:], aps["yk"][:, 0:KP * 2 * GRP])

        env = dict(locals())
        if loop_iters > 1:
            # two reps per hardware iteration with alternating d buffers:
            # rep B writes d_B while ACT still exps over rep A's d_A, so
            # phase C never blocks the next rep's DVE/PE pipeline.
            with tc.For_i(0, loop_iters // 2, 1,
                          hint_engines=(mybir.EngineType.PE,),
                          staggered_reset=True):
                _emit_rep(tc, nc, aps, env, d_A, "a")
                _emit_rep(tc, nc, aps, env, d_B, "b")
        else:
            _emit_rep(tc, nc, aps, env, d_A, "a")


def build_program(loop_iters=1):
    key = ("prog", loop_iters, VARIANT)
    if key in _PROGRAM_CACHE:
        return _PROGRAM_CACHE[key]

    import concourse.tile as tile
    from concourse import bacc, mybir

    f32 = mybir.dt.float32
    bf16 = mybir.dt.bfloat16
    fp8 = mybir.dt.float8e4

    nc = bacc.Bacc("TRN2", target_bir_lowering=False, debug=False,
                   enable_asserts=False, num_devices=N_CORES)
    aps = {
        "xk": nc.dram_tensor("xk", [CH, KP * 2 * TPC], fp8,
                             kind="ExternalInput").ap(),
        "yk": nc.dram_tensor("yk", [CH, NG * KP * 2 * GRP], fp8,
                             kind="ExternalInput").ap(),
        "xm": nc.dram_tensor("xm", [CH, 2 * TPC], fp8,
                             kind="ExternalInput").ap(),
        "ym": nc.dram_tensor("ym", [CH, 2 * NR], fp8,
                             kind="ExternalInput").ap(),
        "rowt": nc.dram_tensor("rowt", [CH, MT], f32,
                               kind="ExternalInput").ap(),
        "souts": nc.dram_tensor("souts", [CH, 2 * MT], f32,
                                kind="ExternalOutput").ap(),
    }
    with tile.TileContext(nc) as tc:
        _build_kernel_body(tc, aps, loop_iters=loop_iters)
    nc.compile()

    _PROGRAM_CACHE[key] = (nc, aps)
    return nc, aps


def host_prepare(target_features, refer_features, mask, target_field,
                 refer_field):
    tgt = np.asarray(target_features)[0].astype(np.float64)
    ref = np.asarray(refer_features)[0].astype(np.float64)
    msk = np.asarray(mask)[0, 0].astype(np.float64)
    t_iy, t_ix = _field_to_idx(target_field)
    r_iy, r_ix = _field_to_idx(refer_field)

    xg = _gather_cols(tgt, t_iy, t_ix).reshape(KC, NT)
    yg = _gather_cols(ref, r_iy, r_ix).reshape(KC, NR)

    y_mean = yg.mean(axis=1, keepdims=True)
    xc = xg - y_mean
    yc = yg - y_mean
    xn = xc / (np.linalg.norm(xc, axis=0, keepdims=True) + EPS)
    yn = yc / (np.linalg.norm(yc, axis=0, keepdims=True) + EPS)

    # fp8 contraction blocks: rows 0..6271 features, 6272 const, rest zero
    Xq = np.zeros((KCP, NT), dtype=E4NP)
    Xq[:KC] = (-ASC * xn).astype(E4NP)
    Xq[KC] = E4NP(ASC)
    Yq = np.zeros((KCP, NR), dtype=E4NP)
    Yq[:KC] = (ASC * yn).astype(E4NP)
    Yq[KC] = E4NP(ASC)

    # yk slab layout: [g][kp][128, 2, GRP]
    yk_arr = np.ascontiguousarray(
        Yq.reshape(KP, 2, CH, NG, GRP).transpose(3, 0, 2, 1, 4)
        .reshape(NG * KP, CH, 2 * GRP).transpose(1, 0, 2).reshape(CH, -1))

    # mask chain
    tp = _gather_cols(msk[None], t_iy, t_ix)[0]   # [49, NT] f64
    rp = _gather_cols(msk[None], r_iy, r_ix)[0]   # [49, NR]
    tpn = (tp ** 2).sum(axis=0)
    rpn = (rp ** 2).sum(axis=0)
    # fp8 DR mask chain: msc*rp_i.tp_j via hi/lo cross terms (196 rows) +
    # (-0.5*msc)*rpn_j via an fp8 ladder (14 rows of x-const C).
    msc = -(2.0 * PROG_W * FSC / K)
    sqs = np.sqrt(-msc)
    MC = 224.0
    LADDER = [8, 2, 2, 2]

    def _f8(v):
        return np.asarray(v, np.float64).astype(E4NP).astype(np.float64)

    vx, vy = -sqs * rp, sqs * tp
    hx = _f8(vx); lx = vx - hx
    hy = _f8(vy); ly = vy - hy
    ym_rows = np.zeros((256, NR), dtype=np.float64)
    ym_rows[0:49] = hy
    ym_rows[49:98] = ly
    ym_rows[98:147] = hy
    ym_rows[147:196] = ly
    t0 = (-0.5 * msc) * rpn
    t = t0.copy(); acc = np.zeros_like(t0); r = 196
    for n in LADDER:
        yv = _f8(t / (n * MC))
        for _ in range(n):
            ym_rows[r] = yv; r += 1
        acc += n * MC * yv; t = t0 - acc
    ym_arr = np.ascontiguousarray(
        ym_rows.reshape(2, CH, NR).transpose(1, 0, 2)).astype(E4NP)

    xm_rows_full = np.zeros((256, NT), dtype=np.float64)
    xm_rows_full[0:49] = hx
    xm_rows_full[49:98] = hx
    xm_rows_full[98:147] = lx
    xm_rows_full[147:196] = lx
    xm_rows_full[196:196 + sum(LADDER)] = MC

    rowt_full = ((PROG_W * FSC / K) * tpn).astype(np.float32)

    in_maps = []
    for c in range(N_CORES):
        rows = slice(c * TPC, (c + 1) * TPC)
        xk_arr = np.ascontiguousarray(
            Xq[:, rows].reshape(KP * 2, CH, TPC).transpose(1, 0, 2)
            .reshape(CH, -1))
        xm_arr = np.ascontiguousarray(
            xm_rows_full[:, rows].reshape(2, CH, TPC)
            .transpose(1, 0, 2)).astype(E4NP)
        rowt_arr = np.ascontiguousarray(
            rowt_full[rows].reshape(MT, CH).T)
        in_maps.append({
            "xk": xk_arr,
            "yk": yk_arr,
            "xm": xm_arr.reshape(CH, -1),
            "ym": ym_arr.reshape(CH, -1),
            "rowt": rowt_arr,
        })
    return in_maps


def finish(stats_list):
    """stats_list: per-core [128, 2*MT] f32 -> scalar loss."""
    losses = np.empty(NT, dtype=np.float64)
    for c, st in enumerate(stats_list):
        st = np.asarray(st, dtype=np.float64)
        for m in range(MT):
            dmin = st[:, 2 * m] / FSC
            sumw = st[:, 2 * m + 1]
            rec = 1.0 / (dmin + EPS)
            losses[c * TPC + m * 128: c * TPC + (m + 1) * 128] = (
                np.log(sumw) - 2.0 * (1.0 - dmin * rec))
    return np.float32(losses.mean())


def kernel(target_features, refer_features, mask, target_field, refer_field):
    from concourse.bass_utils import run_bass_kernel_spmd

    nc, _ = build_program()
    in_maps = host_prepare(target_features, refer_features, mask,
                           target_field, refer_field)
    res = run_bass_kernel_spmd(nc, in_maps, core_ids=list(range(N_CORES)))
    stats_list = [r["souts"] for r in res.results]
    return finish(stats_list)


if __name__ == "__main__":
    rng = np.random.default_rng(0)
    inputs = {
        "target_features": rng.random((1, 128, 256, 256), dtype=np.float32),
        "refer_features": rng.random((1, 128, 256, 256), dtype=np.float32),
        "mask": rng.random((1, 1, 256, 256), dtype=np.float32),
        "target_field": (rng.random((1, 64, 64, 2), dtype=np.float32) * 2 - 1),
        "refer_field": (rng.random((1, 64, 64, 2), dtype=np.float32) * 2 - 1),
    }
    out = kernel(**inputs)
    print("kernel loss:", out)


# revision 32
# speedup vs baseline: 1.0127x; 1.0127x over previous
"""Trainium2 Bass kernel for nn_GuidedCorrespondenceLoss (fp8 DoubleRow).

Strategy (8 NeuronCores, SPMD, target rows sharded 512/core):
  - Host: nearest-neighbor fields -> patch indices; gather 7x7x128 patch
    columns; center by refer-mean and normalize in f64; quantize the
    normalized features to fp8 e4m3 scaled by A=128. The cosine-distance
    matmul runs entirely in fp8 DoubleRow mode (2 contraction rows per
    partition): Pf = F*d_cos with F = 2*A^2 = 32768, via 25 DR k-pairs
    (6272 feat rows + one 128*128 const row pair = 0.5F + zero padding).
  - PE microarch rules (measured): a matmul whose moving operand exceeds
    512 bytes/partition stalls back-to-back streaming (~1.2 cy/col vs
    1.0), and two accumulation chains sharing one PSUM bank serialize.
    So: 256-col fp8 DR chunks (2B/partition/col), column groups of 512
    with 8 concurrent chains in 8 distinct banks (half-bank used,
    ping-pong halves across groups to hide the drain).
  - Mask L2 term (reference broadcast quirk: d_prog[i,j] = tpn_i + rpn_j
    - 2*tp_j.rp_i, clipped at 0 -- the clip is materially active) runs as
    an upfront bf16 phase: Pm = rp^T tp - 0.5*(rpn_hi + rpn_lo) with the
    rpn hi/lo split giving ~1e-5 relative precision on the const term.
    v' = relu(-(20F/49)*Pm + (10F/49)*tpn_i) on ACT writes d_t directly.
  - Feature chains accumulate in PSUM per (m-tile, 256-col chunk); d_t +=
    Pf on DVE; per-(m,group) partial min; per-m: dmin, exp row-sum
    (relaxed contextual loss stats).
  - Host: loss_i = log(sumw_i) - 2*(1 - dmin_i/(dmin_i+EPS)); mean.

Numerics validated host-side: rel err ~1.4e-4 vs f64 reference (gate 2e-2).
"""
import numpy as np
import ml_dtypes

EPS = 2.220446049250313e-16
PATCH, STRIDE, SAMPLE, H_BW, PROG_W = 7, 3, 64, 0.5, 10.0
HO = (256 - PATCH) // STRIDE + 1     # 84

N_CORES = 8
NT = 4096              # total target rows
TPC = NT // N_CORES    # 512 rows per core
MT = TPC // 128        # 4 m-tiles per core
NR = 4096              # refer columns
GRP = 256              # columns per feature group
NG = NR // GRP         # 16 groups
BANK = 512             # psum bank width (f32)
CHK = 256              # matmul moving-chunk columns
K = PATCH * PATCH      # 49
CH = 128
KC = CH * K            # 6272 contraction rows
KP = 25                # DoubleRow k-pairs (6400 rows padded)
KCP = KP * 256         # 6400
KM = K + 2             # mask chain contraction (49 + rpn hi/lo rows)
FSC = 32768.0          # F = 2 * A^2
ASC = 128.0            # feature quantization scale

E4NP = ml_dtypes.float8_e4m3
BFNP = ml_dtypes.bfloat16

_PROGRAM_CACHE = {}
import os
VARIANT = os.environ.get("KERNEL_VARIANT", "full")


def _field_to_idx(field):
    g = np.asarray(field)[0].reshape(-1, 2).astype(np.float32)
    gx = (g[:, 0] + np.float32(1.0)) * np.float32(0.5) * np.float32(HO - 1)
    gy = (g[:, 1] + np.float32(1.0)) * np.float32(0.5) * np.float32(HO - 1)
    ix = np.clip(np.round(gx), 0, HO - 1).astype(np.int64)
    iy = np.clip(np.round(gy), 0, HO - 1).astype(np.int64)
    return iy, ix


def _gather_cols(feat_chw, iy, ix):
    """feat [C,256,256] -> [C, 49, n]: out[c,k,j] = feat[c, 3*iy+kh, 3*ix+kw]."""
    iy3, ix3 = iy * STRIDE, ix * STRIDE
    kh = np.repeat(np.arange(PATCH), PATCH)
    kw = np.tile(np.arange(PATCH), PATCH)
    HH = iy3[None, :] + kh[:, None]
    WW = ix3[None, :] + kw[:, None]
    return feat_chw[:, HH, WW]


def _emit_rep(tc, nc, aps, env, d_t, sfx):
    from concourse import mybir

    f32 = mybir.dt.float32
    bf16 = mybir.dt.bfloat16
    fp8 = mybir.dt.float8e4
    AF = mybir.ActivationFunctionType
    OP = mybir.AluOpType
    DR = mybir.MatmulPerfMode.DoubleRow

    yk, souts = aps["yk"], aps["souts"]
    x_t, xm_t, ym_t, rowt_t, cst = (
        env["x_t"], env["xm_t"], env["ym_t"], env["rowt_t"], env["cst"])
    ypool, wpool, spool, ps = (
        env["ypool"], env["wpool"], env["spool"], env["ps"])

    minp = spool.tile([CH, MT * NG], f32, name=f"minp_{sfx}", tag="minp")

    # ---- per-group: mask block (4 banks rotating) + feature chains (4
    # dedicated full-width banks) + DVE drains.  The mask matmuls at each
    # group head cover the PE window while DVE drains the previous group's
    # chains; the DVE relu (mkl pre-scaled by -(2*PROG_W*FSC/K) on host:
    # d = max(Pm' + tpn'_i, 0)) keeps ACT free for phase C's exp, which
    # then overlaps the next rep's start.
    nodma = VARIANT == "mmnodma"
    KH = (KP + 1) // 2  # y DMA half-group granularity (13 + 12 k-pairs)
    pfs = [env["psf"].tile([CH, MT, GRP], f32, name=f"pf_{p}")
           for p in range(2)]
    for g in range(NG):
        pf = pfs[g % 2]
        if nodma:
            yh = env["yh_c"]
        else:
            yh = ypool.tile([CH, KP, 2, GRP], fp8, name=f"yh_{g}{sfx}",
                            tag="y")
            nc.sync.dma_start(
                yh[:, 0:KH],
                yk[:, (g * KP) * 2 * GRP: (g * KP + KH) * 2 * GRP])
            nc.sync.dma_start(
                yh[:, KH:KP],
                yk[:, (g * KP + KH) * 2 * GRP: (g + 1) * KP * 2 * GRP])
        if VARIANT in ("full", "nophc"):
            for m in range(MT):
                pm = ps.tile([CH, GRP], f32, name=f"pm_{g}_{m}{sfx}",
                             tag="ps")
                nc.tensor.matmul(
                    pm[:],
                    lhsT=xm_t[:, :, m * 128:(m + 1) * 128],
                    rhs=ym_t[:, :, g * GRP:(g + 1) * GRP],
                    start=True, stop=True,
                    perf_mode=DR,
                )
                nc.vector.tensor_scalar(
                    out=d_t[:, m * NR + g * GRP: m * NR + (g + 1) * GRP],
                    in0=pm[:], scalar1=rowt_t[:, m:m + 1], scalar2=0.0,
                    op0=OP.add, op1=OP.max)
        if VARIANT == "dma":
            for m in range(MT):
                nc.tensor.matmul(
                    pf[:, m, :],
                    lhsT=x_t[:, 0:2, m * 128:(m + 1) * 128],
                    rhs=yh[:, 0, :, :],
                    start=True, stop=True,
                    perf_mode=DR,
                )
        else:
          kps = list(range(KP)) if VARIANT != "mmhalf" else list(range(0, KP, 2))
          for kp in kps:
            for m in range(MT):
                nc.tensor.matmul(
                    pf[:, m, :],
                    lhsT=x_t[:, 2 * kp:2 * kp + 2, m * 128:(m + 1) * 128],
                    rhs=yh[:, kp, :, :],
                    start=(kp == kps[0]), stop=(kp == kps[-1]),
                    perf_mode=DR,
                )
        for m in range(MT):
            dsl = d_t[:, m * NR + g * GRP: m * NR + (g + 1) * GRP]
            if VARIANT in ("full", "peadd", "nophc", "nomask"):
                nc.vector.tensor_tensor(
                    out=dsl[:], in0=dsl[:], in1=pf[:, m, :], op=OP.add)
                nc.vector.tensor_reduce(
                    minp[:, m * NG + g: m * NG + g + 1], dsl,
                    axis=mybir.AxisListType.X, op=OP.min)
            else:
                nc.vector.tensor_copy(
                    minp[:, m * NG + g: m * NG + g + 1], pf[:, m, 0:1])

    # ---- Phase C: row stats ---------------------------------------------
    so = spool.tile([CH, 2 * MT], f32, name=f"so_{sfx}", tag="so")
    if VARIANT not in ("full", "nomask"):
        nc.vector.tensor_copy(so[:, 0:MT], minp[:, 0:MT])
        nc.vector.tensor_copy(so[:, MT:2 * MT], minp[:, 0:MT])
        nc.vector.dma_start(souts[:], so[:])
        return
    # dmin and the exp row-sum land directly in the output tile: no DVE
    # copies behind the ACT accum, so the next rep's DVE work (and with it
    # the PE's PSUM rotation) never queues behind this rep's exp.
    for m in range(MT):
        dmin = so[:, 2 * m:2 * m + 1]
        nc.vector.tensor_reduce(dmin, minp[:, m * NG:(m + 1) * NG],
                                axis=mybir.AxisListType.X, op=OP.min)
        dme = spool.tile([CH, 1], f32, name=f"dme_{m}{sfx}", tag="st")
        nc.vector.tensor_scalar(
            out=dme[:], in0=dmin, scalar1=float(1.0 / FSC),
            scalar2=float(EPS), op0=OP.mult, op1=OP.add)
        rec = spool.tile([CH, 1], f32, name=f"rec_{m}{sfx}", tag="st")
        nc.vector.reciprocal(rec[:], dme[:])
        scl = spool.tile([CH, 1], f32, name=f"scl_{m}{sfx}", tag="st")
        nc.vector.tensor_scalar_mul(scl[:], rec[:], float(-2.0 / FSC))
        wtmp = wpool.tile([CH, NR], bf16, name=f"w_{m}{sfx}", tag="w")
        nc.scalar.activation(wtmp[:], d_t[:, m * NR:(m + 1) * NR], AF.Exp,
                             bias=cst[:, 0:1], scale=scl[:, 0:1],
                             accum_out=so[:, 2 * m + 1:2 * m + 2])

    # issue from ACT's queue: it waits on exp(m3) anyway, and must not
    # block the SP queue that streams the next rep's y slabs.
    nc.scalar.dma_start(souts[:], so[:])


def _build_kernel_body(tc, aps, loop_iters=1):
    from concourse import mybir

    nc = tc.nc
    f32 = mybir.dt.float32
    bf16 = mybir.dt.bfloat16
    fp8 = mybir.dt.float8e4

    with (
        tc.tile_pool(name="xpool", bufs=1) as xpool,
        tc.tile_pool(name="dpool", bufs=1) as dpool,
        tc.tile_pool(name="ypool", bufs=3) as ypool,
        tc.tile_pool(name="wpool", bufs=2) as wpool,
        tc.tile_pool(name="spool", bufs=4) as spool,
        tc.tile_pool(name="ps", bufs=4, space="PSUM") as ps,
        tc.tile_pool(name="psf", bufs=1, space="PSUM") as psf,
    ):
        # resident loads (outside the timing loop)
        x_t = xpool.tile([CH, KP * 2, TPC], fp8, name="x_t")
        nc.sync.dma_start(x_t[:], aps["xk"][:])
        xm_t = xpool.tile([CH, 2, TPC], fp8, name="xm_t")
        nc.sync.dma_start(xm_t[:], aps["xm"][:])
        ym_t = xpool.tile([CH, 2, NR], fp8, name="ym_t")
        nc.sync.dma_start(ym_t[:], aps["ym"][:])
        rowt_t = xpool.tile([CH, MT], f32, name="rowt_t")
        nc.sync.dma_start(rowt_t[:], aps["rowt"][:])
        cst = xpool.tile([CH, 1], f32, name="cst")
        nc.vector.memset(cst[:, 0:1], 2.0)

        d_A = dpool.tile([CH, MT * NR], bf16, name="d_A")
        d_B = dpool.tile([CH, MT * NR], bf16, name="d_B")

        if VARIANT == "mmnodma":
            yh_c = xpool.tile([CH, KP, 2, GRP], fp8, name="yh_c")
            nc.sync.dma_start(yh_c[:], aps["yk"][:, 0:KP * 2 * GRP])

        env = dict(locals())
        if loop_iters > 1:
            # two reps per hardware iteration with alternating d buffers:
            # rep B writes d_B while ACT still exps over rep A's d_A, so
            # phase C never blocks the next rep's DVE/PE pipeline.
            with tc.For_i(0, loop_iters // 2, 1,
                          hint_engines=(mybir.EngineType.PE,),
                          staggered_reset=True):
                _emit_rep(tc, nc, aps, env, d_A, "a")
                _emit_rep(tc, nc, aps, env, d_B, "b")
        else:
            _emit_rep(tc, nc, aps, env, d_A, "a")


def build_program(loop_iters=1):
    key = ("prog", loop_iters, VARIANT)
    if key in _PROGRAM_CACHE:
        return _PROGRAM_CACHE[key]

    import concourse.tile as tile
    from concourse import bacc, mybir

    f32 = mybir.dt.float32
    bf16 = mybir.dt.bfloat16
    fp8 = mybir.dt.float8e4

    nc = bacc.Bacc("TRN2", target_bir_lowering=False, debug=False,
                   enable_asserts=False, num_devices=N_CORES)
    aps = {
        "xk": nc.dram_tensor("xk", [CH, KP * 2 * TPC], fp8,
                             kind="ExternalInput").ap(),
        "yk": nc.dram_tensor("yk", [CH, NG * KP * 2 * GRP], fp8,
                             kind="ExternalInput").ap(),
        "xm": nc.dram_tensor("xm", [CH, 2 * TPC], fp8,
                             kind="ExternalInput").ap(),
        "ym": nc.dram_tensor("ym", [CH, 2 * NR], fp8,
                             kind="ExternalInput").ap(),
        "rowt": nc.dram_tensor("rowt", [CH, MT], f32,
                               kind="ExternalInput").ap(),
        "souts": nc.dram_tensor("souts", [CH, 2 * MT], f32,
                                kind="ExternalOutput").ap(),
    }
    with tile.TileContext(nc) as tc:
        _build_kernel_body(tc, aps, loop_iters=loop_iters)
    nc.compile()

    _PROGRAM_CACHE[key] = (nc, aps)
    return nc, aps


def host_prepare(target_features, refer_features, mask, target_field,
                 refer_field):
    tgt = np.asarray(target_features)[0].astype(np.float64)
    ref = np.asarray(refer_features)[0].astype(np.float64)
    msk = np.asarray(mask)[0, 0].astype(np.float64)
    t_iy, t_ix = _field_to_idx(target_field)
    r_iy, r_ix = _field_to_idx(refer_field)

    xg = _gather_cols(tgt, t_iy, t_ix).reshape(KC, NT)
    yg = _gather_cols(ref, r_iy, r_ix).reshape(KC, NR)

    y_mean = yg.mean(axis=1, keepdims=True)
    xc = xg - y_mean
    yc = yg - y_mean
    xn = xc / (np.linalg.norm(xc, axis=0, keepdims=True) + EPS)
    yn = yc / (np.linalg.norm(yc, axis=0, keepdims=True) + EPS)

    # fp8 contraction blocks: rows 0..6271 features, 6272 const, rest zero
    Xq = np.zeros((KCP, NT), dtype=E4NP)
    Xq[:KC] = (-ASC * xn).astype(E4NP)
    Xq[KC] = E4NP(ASC)
    Yq = np.zeros((KCP, NR), dtype=E4NP)
    Yq[:KC] = (ASC * yn).astype(E4NP)
    Yq[KC] = E4NP(ASC)

    # yk slab layout: [g][kp][128, 2, GRP]
    yk_arr = np.ascontiguousarray(
        Yq.reshape(KP, 2, CH, NG, GRP).transpose(3, 0, 2, 1, 4)
        .reshape(NG * KP, CH, 2 * GRP).transpose(1, 0, 2).reshape(CH, -1))

    # mask chain
    tp = _gather_cols(msk[None], t_iy, t_ix)[0]   # [49, NT] f64
    rp = _gather_cols(msk[None], r_iy, r_ix)[0]   # [49, NR]
    tpn = (tp ** 2).sum(axis=0)
    rpn = (rp ** 2).sum(axis=0)
    # fp8 DR mask chain: msc*rp_i.tp_j via hi/lo cross terms (196 rows) +
    # (-0.5*msc)*rpn_j via an fp8 ladder (14 rows of x-const C).
    msc = -(2.0 * PROG_W * FSC / K)
    sqs = np.sqrt(-msc)
    MC = 224.0
    LADDER = [8, 2, 2, 2]

    def _f8(v):
        return np.asarray(v, np.float64).astype(E4NP).astype(np.float64)

    vx, vy = -sqs * rp, sqs * tp
    hx = _f8(vx); lx = vx - hx
    hy = _f8(vy); ly = vy - hy
    ym_rows = np.zeros((256, NR), dtype=np.float64)
    ym_rows[0:49] = hy
    ym_rows[49:98] = ly
    ym_rows[98:147] = hy
    ym_rows[147:196] = ly
    t0 = (-0.5 * msc) * rpn
    t = t0.copy(); acc = np.zeros_like(t0); r = 196
    for n in LADDER:
        yv = _f8(t / (n * MC))
        for _ in range(n):
            ym_rows[r] = yv; r += 1
        acc += n * MC * yv; t = t0 - acc
    ym_arr = np.ascontiguousarray(
        ym_rows.reshape(2, CH, NR).transpose(1, 0, 2)).astype(E4NP)

    xm_rows_full = np.zeros((256, NT), dtype=np.float64)
    xm_rows_full[0:49] = hx
    xm_rows_full[49:98] = hx
    xm_rows_full[98:147] = lx
    xm_rows_full[147:196] = lx
    xm_rows_full[196:196 + sum(LADDER)] = MC

    rowt_full = ((PROG_W * FSC / K) * tpn).astype(np.float32)

    in_maps = []
    for c in range(N_CORES):
        rows = slice(c * TPC, (c + 1) * TPC)
        xk_arr = np.ascontiguousarray(
            Xq[:, rows].reshape(KP * 2, CH, TPC).transpose(1, 0, 2)
            .reshape(CH, -1))
        xm_arr = np.ascontiguousarray(
            xm_rows_full[:, rows].reshape(2, CH, TPC)
            .transpose(1, 0, 2)).astype(E4NP)
        rowt_arr = np.ascontiguousarray(
            rowt_full[rows].reshape(MT, CH).T)
        in_maps.append({
            "xk": xk_arr,
            "yk": yk_arr,
            "xm": xm_arr.reshape(CH, -1),
            "ym": ym_arr.reshape(CH, -1),
            "rowt": rowt_arr,
        })
    return in_maps


def finish(stats_list):
    """stats_list: per-core [128, 2*MT] f32 -> scalar loss."""
    losses = np.empty(NT, dtype=np.float64)
    for c, st in enumerate(stats_list):
        st = np.asarray(st, dtype=np.float64)
        for m in range(MT):
            dmin = st[:, 2 * m] / FSC
            sumw = st[:, 2 * m + 1]
            rec = 1.0 / (dmin + EPS)
            losses[c * TPC + m * 128: c * TPC + (m + 1) * 128] = (
                np.log(sumw) - 2.0 * (1.0 - dmin * rec))
    return np.float32(losses.mean())


def kernel(target_features, refer_features, mask, target_field, refer_field):
    from concourse.bass_utils import run_bass_kernel_spmd

    nc, _ = build_program()
    in_maps = host_prepare(target_features, refer_features, mask,
                           target_field, refer_field)
    res = run_bass_kernel_spmd(nc, in_maps, core_ids=list(range(N_CORES)))
    stats_list = [r["souts"] for r in res.results]
    return finish(stats_list)


if __name__ == "__main__":
    rng = np.random.default_rng(0)
    inputs = {
        "target_features": rng.random((1, 128, 256, 256), dtype=np.float32),
        "refer_features": rng.random((1, 128, 256, 256), dtype=np.float32),
        "mask": rng.random((1, 1, 256, 256), dtype=np.float32),
        "target_field": (rng.random((1, 64, 64, 2), dtype=np.float32) * 2 - 1),
        "refer_field": (rng.random((1, 64, 64, 2), dtype=np.float32) * 2 - 1),
    }
    out = kernel(**inputs)
    print("kernel loss:", out)


# revision 36
# speedup vs baseline: 1.0250x; 1.0122x over previous
"""Trainium2 Bass kernel for nn_GuidedCorrespondenceLoss (fp8 DoubleRow).

Strategy (8 NeuronCores, SPMD, target rows sharded 512/core):
  - Host: nearest-neighbor fields -> patch indices; gather 7x7x128 patch
    columns; center by refer-mean and normalize in f64; quantize the
    normalized features to fp8 e4m3 scaled by A=128. The cosine-distance
    matmul runs entirely in fp8 DoubleRow mode (2 contraction rows per
    partition): Pf = F*d_cos with F = 2*A^2 = 32768, via 25 DR k-pairs
    (6272 feat rows + one 128*128 const row pair = 0.5F + zero padding).
  - PE microarch rules (measured): a matmul whose moving operand exceeds
    512 bytes/partition stalls back-to-back streaming (~1.2 cy/col vs
    1.0), and two accumulation chains sharing one PSUM bank serialize.
    So: 256-col fp8 DR chunks (2B/partition/col), column groups of 512
    with 8 concurrent chains in 8 distinct banks (half-bank used,
    ping-pong halves across groups to hide the drain).
  - Mask L2 term (reference broadcast quirk: d_prog[i,j] = tpn_i + rpn_j
    - 2*tp_j.rp_i, clipped at 0 -- the clip is materially active) runs as
    an upfront bf16 phase: Pm = rp^T tp - 0.5*(rpn_hi + rpn_lo) with the
    rpn hi/lo split giving ~1e-5 relative precision on the const term.
    v' = relu(-(20F/49)*Pm + (10F/49)*tpn_i) on ACT writes d_t directly.
  - Feature chains accumulate in PSUM per (m-tile, 256-col chunk); d_t +=
    Pf on DVE; per-(m,group) partial min; per-m: dmin, exp row-sum
    (relaxed contextual loss stats).
  - Host: loss_i = log(sumw_i) - 2*(1 - dmin_i/(dmin_i+EPS)); mean.

Numerics validated host-side: rel err ~1.4e-4 vs f64 reference (gate 2e-2).
"""
import numpy as np
import ml_dtypes

EPS = 2.220446049250313e-16
PATCH, STRIDE, SAMPLE, H_BW, PROG_W = 7, 3, 64, 0.5, 10.0
HO = (256 - PATCH) // STRIDE + 1     # 84

N_CORES = 8
NT = 4096              # total target rows
TPC = NT // N_CORES    # 512 rows per core
MT = TPC // 128        # 4 m-tiles per core
NR = 4096              # refer columns
GRP = 256              # columns per feature group
NG = NR // GRP         # 16 groups
BANK = 512             # psum bank width (f32)
CHK = 256              # matmul moving-chunk columns
K = PATCH * PATCH      # 49
CH = 128
KC = CH * K            # 6272 contraction rows
KP = 25                # DoubleRow k-pairs (6400 rows padded)
KCP = KP * 256         # 6400
KM = K + 2             # mask chain contraction (49 + rpn hi/lo rows)
FSC = 32768.0          # F = 2 * A^2
ASC = 128.0            # feature quantization scale

E4NP = ml_dtypes.float8_e4m3
BFNP = ml_dtypes.bfloat16

_PROGRAM_CACHE = {}
import os
VARIANT = os.environ.get("KERNEL_VARIANT", "full")


def _field_to_idx(field):
    g = np.asarray(field)[0].reshape(-1, 2).astype(np.float32)
    gx = (g[:, 0] + np.float32(1.0)) * np.float32(0.5) * np.float32(HO - 1)
    gy = (g[:, 1] + np.float32(1.0)) * np.float32(0.5) * np.float32(HO - 1)
    ix = np.clip(np.round(gx), 0, HO - 1).astype(np.int64)
    iy = np.clip(np.round(gy), 0, HO - 1).astype(np.int64)
    return iy, ix


def _gather_cols(feat_chw, iy, ix):
    """feat [C,256,256] -> [C, 49, n]: out[c,k,j] = feat[c, 3*iy+kh, 3*ix+kw]."""
    iy3, ix3 = iy * STRIDE, ix * STRIDE
    kh = np.repeat(np.arange(PATCH), PATCH)
    kw = np.tile(np.arange(PATCH), PATCH)
    HH = iy3[None, :] + kh[:, None]
    WW = ix3[None, :] + kw[:, None]
    return feat_chw[:, HH, WW]


def _emit_rep(tc, nc, aps, env, d_t, sfx):
    from concourse import mybir

    f32 = mybir.dt.float32
    bf16 = mybir.dt.bfloat16
    fp8 = mybir.dt.float8e4
    AF = mybir.ActivationFunctionType
    OP = mybir.AluOpType
    DR = mybir.MatmulPerfMode.DoubleRow

    yk, souts = aps["yk"], aps["souts"]
    x_t, xm_t, ym_t, rowt_t, cst = (
        env["x_t"], env["xm_t"], env["ym_t"], env["rowt_t"], env["cst"])
    ypool, wpool, spool, ps = (
        env["ypool"], env["wpool"], env["spool"], env["ps"])

    minp = spool.tile([CH, MT * NG], f32, name=f"minp_{sfx}", tag="minp")

    # ---- per-group: mask block (4 banks rotating) + feature chains (4
    # dedicated full-width banks) + DVE drains.  The mask matmuls at each
    # group head cover the PE window while DVE drains the previous group's
    # chains; the DVE relu (mkl pre-scaled by -(2*PROG_W*FSC/K) on host:
    # d = max(Pm' + tpn'_i, 0)) keeps ACT free for phase C's exp, which
    # then overlaps the next rep's start.
    nodma = VARIANT == "mmnodma"
    KH = (KP + 1) // 2  # y DMA half-group granularity (13 + 12 k-pairs)
    pfs = [env["psf"].tile([CH, MT, GRP], f32, name=f"pf_{p}")
           for p in range(2)]
    for g in range(NG):
        pf = pfs[g % 2]
        if nodma:
            yh = env["yh_c"]
        else:
            yh = ypool.tile([CH, KP, 2, GRP], fp8, name=f"yh_{g}{sfx}",
                            tag="y")
            nc.sync.dma_start(
                yh[:, 0:KH],
                yk[:, (g * KP) * 2 * GRP: (g * KP + KH) * 2 * GRP])
            nc.sync.dma_start(
                yh[:, KH:KP],
                yk[:, (g * KP + KH) * 2 * GRP: (g + 1) * KP * 2 * GRP])
        if VARIANT in ("full", "nophc"):
            for m in range(MT):
                pm = ps.tile([CH, GRP], f32, name=f"pm_{g}_{m}{sfx}",
                             tag="ps")
                nc.tensor.matmul(
                    pm[:],
                    lhsT=xm_t[:, :, m * 128:(m + 1) * 128],
                    rhs=ym_t[:, :, g * GRP:(g + 1) * GRP],
                    start=True, stop=True,
                    perf_mode=DR,
                )
                nc.vector.tensor_scalar(
                    out=d_t[:, m * NR + g * GRP: m * NR + (g + 1) * GRP],
                    in0=pm[:], scalar1=rowt_t[:, m:m + 1], scalar2=0.0,
                    op0=OP.add, op1=OP.max)
        if VARIANT == "dma":
            for m in range(MT):
                nc.tensor.matmul(
                    pf[:, m, :],
                    lhsT=x_t[:, 0:2, m * 128:(m + 1) * 128],
                    rhs=yh[:, 0, :, :],
                    start=True, stop=True,
                    perf_mode=DR,
                )
        else:
          kps = list(range(KP)) if VARIANT != "mmhalf" else list(range(0, KP, 2))
          for kp in kps:
            for m in range(MT):
                nc.tensor.matmul(
                    pf[:, m, :],
                    lhsT=x_t[:, 2 * kp:2 * kp + 2, m * 128:(m + 1) * 128],
                    rhs=yh[:, kp, :, :],
                    start=(kp == kps[0]), stop=(kp == kps[-1]),
                    perf_mode=DR,
                )
        for m in range(MT):
            dsl = d_t[:, m * NR + g * GRP: m * NR + (g + 1) * GRP]
            if VARIANT in ("full", "peadd", "nophc", "nomask"):
                nc.vector.tensor_tensor(
                    out=dsl[:], in0=dsl[:], in1=pf[:, m, :], op=OP.add)
                nc.vector.tensor_reduce(
                    minp[:, m * NG + g: m * NG + g + 1], dsl,
                    axis=mybir.AxisListType.X, op=OP.min)
            else:
                nc.vector.tensor_copy(
                    minp[:, m * NG + g: m * NG + g + 1], pf[:, m, 0:1])

    # ---- Phase C: row stats ---------------------------------------------
    so = spool.tile([CH, 2 * MT], f32, name=f"so_{sfx}", tag="so")
    if VARIANT not in ("full", "nomask"):
        nc.vector.tensor_copy(so[:, 0:MT], minp[:, 0:MT])
        nc.vector.tensor_copy(so[:, MT:2 * MT], minp[:, 0:MT])
        nc.sync.dma_start(souts[:], so[:])
        return
    # dmin and the exp row-sum land directly in the output tile: no DVE
    # copies behind the ACT accum, so the next rep's DVE work (and with it
    # the PE's PSUM rotation) never queues behind this rep's exp.
    for m in range(MT):
        dmin = so[:, 2 * m:2 * m + 1]
        nc.vector.tensor_reduce(dmin, minp[:, m * NG:(m + 1) * NG],
                                axis=mybir.AxisListType.X, op=OP.min)
        dme = spool.tile([CH, 1], f32, name=f"dme_{m}{sfx}", tag="st")
        nc.vector.tensor_scalar(
            out=dme[:], in0=dmin, scalar1=float(1.0 / FSC),
            scalar2=float(EPS), op0=OP.mult, op1=OP.add)
        rec = spool.tile([CH, 1], f32, name=f"rec_{m}{sfx}", tag="st")
        nc.vector.reciprocal(rec[:], dme[:])
        scl = spool.tile([CH, 1], f32, name=f"scl_{m}{sfx}", tag="st")
        nc.vector.tensor_scalar_mul(scl[:], rec[:], float(-2.0 / FSC))
        wtmp = wpool.tile([CH, NR], bf16, name=f"w_{m}{sfx}", tag="w")
        nc.scalar.activation(wtmp[:], d_t[:, m * NR:(m + 1) * NR], AF.Exp,
                             bias=cst[:, 0:1], scale=scl[:, 0:1],
                             accum_out=so[:, 2 * m + 1:2 * m + 2])

    # issue from ACT's queue: it waits on exp(m3) anyway, and must not
    # block the SP queue that streams the next rep's y slabs.
    nc.scalar.dma_start(souts[:], so[:])


def _build_kernel_body(tc, aps, loop_iters=1):
    from concourse import mybir

    nc = tc.nc
    f32 = mybir.dt.float32
    bf16 = mybir.dt.bfloat16
    fp8 = mybir.dt.float8e4

    with (
        tc.tile_pool(name="xpool", bufs=1) as xpool,
        tc.tile_pool(name="dpool", bufs=1) as dpool,
        tc.tile_pool(name="ypool", bufs=3) as ypool,
        tc.tile_pool(name="wpool", bufs=2) as wpool,
        tc.tile_pool(name="spool", bufs=4) as spool,
        tc.tile_pool(name="ps", bufs=4, space="PSUM") as ps,
        tc.tile_pool(name="psf", bufs=1, space="PSUM") as psf,
    ):
        # resident loads (outside the timing loop)
        x_t = xpool.tile([CH, KP * 2, TPC], fp8, name="x_t")
        nc.sync.dma_start(x_t[:], aps["xk"][:])
        xm_t = xpool.tile([CH, 2, TPC], fp8, name="xm_t")
        nc.sync.dma_start(xm_t[:], aps["xm"][:])
        ym_t = xpool.tile([CH, 2, NR], fp8, name="ym_t")
        nc.sync.dma_start(ym_t[:], aps["ym"][:])
        rowt_t = xpool.tile([CH, MT], f32, name="rowt_t")
        nc.sync.dma_start(rowt_t[:], aps["rowt"][:])
        cst = xpool.tile([CH, 1], f32, name="cst")
        nc.vector.memset(cst[:, 0:1], 2.0)

        d_A = dpool.tile([CH, MT * NR], bf16, name="d_A")
        d_B = dpool.tile([CH, MT * NR], bf16, name="d_B")

        if VARIANT == "mmnodma":
            yh_c = xpool.tile([CH, KP, 2, GRP], fp8, name="yh_c")
            nc.sync.dma_start(yh_c[:], aps["yk"][:, 0:KP * 2 * GRP])

        env = dict(locals())
        if loop_iters > 1:
            # two reps per hardware iteration with alternating d buffers:
            # rep B writes d_B while ACT still exps over rep A's d_A, so
            # phase C never blocks the next rep's DVE/PE pipeline.
            with tc.For_i(0, loop_iters // 2, 1,
                          hint_engines=(mybir.EngineType.PE,),
                          staggered_reset=True):
                _emit_rep(tc, nc, aps, env, d_A, "a")
                _emit_rep(tc, nc, aps, env, d_B, "b")
        else:
            _emit_rep(tc, nc, aps, env, d_A, "a")


def build_program(loop_iters=1):
    key = ("prog", loop_iters, VARIANT)
    if key in _PROGRAM_CACHE:
        return _PROGRAM_CACHE[key]

    import concourse.tile as tile
    from concourse import bacc, mybir

    f32 = mybir.dt.float32
    bf16 = mybir.dt.bfloat16
    fp8 = mybir.dt.float8e4

    nc = bacc.Bacc("TRN2", target_bir_lowering=False, debug=False,
                   enable_asserts=False, num_devices=N_CORES)
    aps = {
        "xk": nc.dram_tensor("xk", [CH, KP * 2 * TPC], fp8,
                             kind="ExternalInput").ap(),
        "yk": nc.dram_tensor("yk", [CH, NG * KP * 2 * GRP], fp8,
                             kind="ExternalInput").ap(),
        "xm": nc.dram_tensor("xm", [CH, 2 * TPC], fp8,
                             kind="ExternalInput").ap(),
        "ym": nc.dram_tensor("ym", [CH, 2 * NR], fp8,
                             kind="ExternalInput").ap(),
        "rowt": nc.dram_tensor("rowt", [CH, MT], f32,
                               kind="ExternalInput").ap(),
        "souts": nc.dram_tensor("souts", [CH, 2 * MT], f32,
                                kind="ExternalOutput").ap(),
    }
    with tile.TileContext(nc) as tc:
        _build_kernel_body(tc, aps, loop_iters=loop_iters)
    nc.compile()

    _PROGRAM_CACHE[key] = (nc, aps)
    return nc, aps


def host_prepare(target_features, refer_features, mask, target_field,
                 refer_field):
    tgt = np.asarray(target_features)[0].astype(np.float64)
    ref = np.asarray(refer_features)[0].astype(np.float64)
    msk = np.asarray(mask)[0, 0].astype(np.float64)
    t_iy, t_ix = _field_to_idx(target_field)
    r_iy, r_ix = _field_to_idx(refer_field)

    xg = _gather_cols(tgt, t_iy, t_ix).reshape(KC, NT)
    yg = _gather_cols(ref, r_iy, r_ix).reshape(KC, NR)

    y_mean = yg.mean(axis=1, keepdims=True)
    xc = xg - y_mean
    yc = yg - y_mean
    xn = xc / (np.linalg.norm(xc, axis=0, keepdims=True) + EPS)
    yn = yc / (np.linalg.norm(yc, axis=0, keepdims=True) + EPS)

    # fp8 contraction blocks: rows 0..6271 features, 6272 const, rest zero
    Xq = np.zeros((KCP, NT), dtype=E4NP)
    Xq[:KC] = (-ASC * xn).astype(E4NP)
    Xq[KC] = E4NP(ASC)
    Yq = np.zeros((KCP, NR), dtype=E4NP)
    Yq[:KC] = (ASC * yn).astype(E4NP)
    Yq[KC] = E4NP(ASC)

    # yk slab layout: [g][kp][128, 2, GRP]
    yk_arr = np.ascontiguousarray(
        Yq.reshape(KP, 2, CH, NG, GRP).transpose(3, 0, 2, 1, 4)
        .reshape(NG * KP, CH, 2 * GRP).transpose(1, 0, 2).reshape(CH, -1))

    # mask chain
    tp = _gather_cols(msk[None], t_iy, t_ix)[0]   # [49, NT] f64
    rp = _gather_cols(msk[None], r_iy, r_ix)[0]   # [49, NR]
    tpn = (tp ** 2).sum(axis=0)
    rpn = (rp ** 2).sum(axis=0)
    # fp8 DR mask chain: msc*rp_i.tp_j via hi/lo cross terms (196 rows) +
    # (-0.5*msc)*rpn_j via an fp8 ladder (14 rows of x-const C).
    msc = -(2.0 * PROG_W * FSC / K)
    sqs = np.sqrt(-msc)
    MC = 224.0
    LADDER = [8, 2, 2, 2]

    def _f8(v):
        return np.asarray(v, np.float64).astype(E4NP).astype(np.float64)

    vx, vy = -sqs * rp, sqs * tp
    hx = _f8(vx); lx = vx - hx
    hy = _f8(vy); ly = vy - hy
    ym_rows = np.zeros((256, NR), dtype=np.float64)
    ym_rows[0:49] = hy
    ym_rows[49:98] = ly
    ym_rows[98:147] = hy
    ym_rows[147:196] = ly
    t0 = (-0.5 * msc) * rpn
    t = t0.copy(); acc = np.zeros_like(t0); r = 196
    for n in LADDER:
        yv = _f8(t / (n * MC))
        for _ in range(n):
            ym_rows[r] = yv; r += 1
        acc += n * MC * yv; t = t0 - acc
    ym_arr = np.ascontiguousarray(
        ym_rows.reshape(2, CH, NR).transpose(1, 0, 2)).astype(E4NP)

    xm_rows_full = np.zeros((256, NT), dtype=np.float64)
    xm_rows_full[0:49] = hx
    xm_rows_full[49:98] = hx
    xm_rows_full[98:147] = lx
    xm_rows_full[147:196] = lx
    xm_rows_full[196:196 + sum(LADDER)] = MC

    rowt_full = ((PROG_W * FSC / K) * tpn).astype(np.float32)

    in_maps = []
    for c in range(N_CORES):
        rows = slice(c * TPC, (c + 1) * TPC)
        xk_arr = np.ascontiguousarray(
            Xq[:, rows].reshape(KP * 2, CH, TPC).transpose(1, 0, 2)
            .reshape(CH, -1))
        xm_arr = np.ascontiguousarray(
            xm_rows_full[:, rows].reshape(2, CH, TPC)
            .transpose(1, 0, 2)).astype(E4NP)
        rowt_arr = np.ascontiguousarray(
            rowt_full[rows].reshape(MT, CH).T)
        in_maps.append({
            "xk": xk_arr,
            "yk": yk_arr,
            "xm": xm_arr.reshape(CH, -1),
            "ym": ym_arr.reshape(CH, -1),
            "rowt": rowt_arr,
        })
    return in_maps


def finish(stats_list):
    """stats_list: per-core [128, 2*MT] f32 -> scalar loss."""
    losses = np.empty(NT, dtype=np.float64)
    for c, st in enumerate(stats_list):
        st = np.asarray(st, dtype=np.float64)
        for m in range(MT):
            dmin = st[:, 2 * m] / FSC
            sumw = st[:, 2 * m + 1]
            rec = 1.0 / (dmin + EPS)
            losses[c * TPC + m * 128: c * TPC + (m + 1) * 128] = (
                np.log(sumw) - 2.0 * (1.0 - dmin * rec))
    return np.float32(losses.mean())


def kernel(target_features, refer_features, mask, target_field, refer_field):
    from concourse.bass_utils import run_bass_kernel_spmd

    nc, _ = build_program()
    in_maps = host_prepare(target_features, refer_features, mask,
                           target_field, refer_field)
    res = run_bass_kernel_spmd(nc, in_maps, core_ids=list(range(N_CORES)))
    stats_list = [r["souts"] for r in res.results]
    return finish(stats_list)


if __name__ == "__main__":
    rng = np.random.default_rng(0)
    inputs = {
        "target_features": rng.random((1, 128, 256, 256), dtype=np.float32),
        "refer_features": rng.random((1, 128, 256, 256), dtype=np.float32),
        "mask": rng.random((1, 1, 256, 256), dtype=np.float32),
        "target_field": (rng.random((1, 64, 64, 2), dtype=np.float32) * 2 - 1),
        "refer_field": (rng.random((1, 64, 64, 2), dtype=np.float32) * 2 - 1),
    }
    out = kernel(**inputs)
    print("kernel loss:", out)
